# revision 8
# baseline (speedup 1.0000x reference)
"""Trainium2 Bass kernel for multi-head attention (nn_Attention_61168924230279).

Module: y = Attention(x) with q/k/v/o linear layers (eqx convention
y = x @ W.T + b), NeoX-style RoPE on q and k, softmax attention.
  x [2, 2048, 2048], 16 heads x 128 head_dim, fp32.

Sharding: tensor-parallel over heads. 8 cores x 2 heads each.
Core c owns feature slice [256c, 256c+256) of wq/wk/wv rows and wo cols.
Each core computes a partial y; partials are summed on the host (+bo).

Device dataflow per core (all matmuls in float32r — full PE rate):
  A) per m-tile (128 rows of x):
       qk_psum = [x@wqkT | bias]  (bias via K=1 ones-row matmul)
       rope applied on DVE (cos/sign-folded-sin tables), then PE-transpose
       per 128-group -> QT/KT [head_dim, m] layout in SBUF; V kept natural.
  B) per (batch, head, 512-query-chunk):
       logitsT[k,q] = KT_tile^T @ QT_chunk  (PE), exp on ACT (psum->sbuf),
       out_T = V^T @ attnT accumulated over key tiles (PE),
       l[q] = ones^T @ attnT (PE), r = 1/l (DVE),
       r broadcast across partitions via K=1 ones matmul,
       outT = av_psum * rb (DVE) -> normalized transposed head outputs.
  C) per (batch, m-tile): y = sum_h outT_h^T @ woT_h (PE), evict via ACT,
       DMA to DRAM partial output.
"""

import math
import os
from contextlib import ExitStack

import numpy as np

import concourse.bass as bass
import concourse.tile as tile
from concourse import bacc, mybir
from concourse.masks import make_identity

P = 128
D = 2048
ND = D // P            # 16 contraction chunks
B = 2
S = 2048
M = B * S              # 4096
NMT = S // P           # 16 m-tiles per batch
HD = 128
HPC = 2                # heads per core
E2 = HPC * HD          # 256 (v width per core)
E4 = 2 * E2            # 512 (q|k width per core)
NQC = S // 512         # 4 query chunks per batch
NKT = S // P           # 16 key tiles per batch
N_CORES = 8
SCALE = 1.0 / math.sqrt(HD)
ROPE_THETA = 10000.0

F32 = mybir.dt.float32
F32R = mybir.dt.float32r


def _emit(nc, tc, t):
    """Emit the per-core program. t: dict of DRAM APs."""
    phases = os.environ.get("K_PHASES", "ABC")
    with ExitStack() as ctx:
        ec = ctx.enter_context
        const = ec(tc.tile_pool(name="const", bufs=1))
        wpool = ec(tc.tile_pool(name="weights", bufs=1))
        tabs = ec(tc.tile_pool(name="tables", bufs=1))
        xtp = ec(tc.tile_pool(name="xt", bufs=2))
        qkp = ec(tc.tile_pool(name="qk", bufs=2))
        up = ec(tc.tile_pool(name="u", bufs=2))
        qtkv = ec(tc.tile_pool(name="qtkv", bufs=1))
        attnp = ec(tc.tile_pool(name="attn", bufs=3))
        rbp = ec(tc.tile_pool(name="rb", bufs=2))
        rsp = ec(tc.tile_pool(name="rs", bufs=2))
        outp = ec(tc.tile_pool(name="outT", bufs=1))
        yp = ec(tc.tile_pool(name="y", bufs=4))
        psA = ec(tc.tile_pool(name="psA", bufs=4, space="PSUM"))
        psAcc = ec(tc.tile_pool(name="psAcc", bufs=2, space="PSUM"))
        psL = ec(tc.tile_pool(name="psL", bufs=2, space="PSUM"))

        # --- constants / weights / tables (loaded once) ---
        ones_r32 = const.tile([1, P], F32)
        nc.vector.memset(ones_r32, 1.0)
        ones_r = const.tile([1, P], F32R)      # ones row: lhsT for broadcasts
        nc.vector.tensor_copy(ones_r, ones_r32)
        ones_c32 = const.tile([P, 1], F32)
        nc.vector.memset(ones_c32, 1.0)
        ones_c = const.tile([P, 1], F32R)      # ones col: lhsT for l-sums
        nc.vector.tensor_copy(ones_c, ones_c32)
        ident = const.tile([P, P], F32)
        make_identity(nc, ident)
        bqk_s = const.tile([1, E4], F32R)
        nc.gpsimd.dma_start(bqk_s, t["bqk"])
        bv_s = const.tile([1, E2], F32R)
        nc.gpsimd.dma_start(bv_s, t["bv"])

        wqk_s = wpool.tile([P, ND, E4], F32R)
        nc.gpsimd.dma_start(wqk_s, t["wqkT"].rearrange("(k p) e -> p k e", p=P))
        wv_s = wpool.tile([P, ND, E2], F32R)
        nc.gpsimd.dma_start(wv_s, t["wvT"].rearrange("(k p) e -> p k e", p=P))
        wo_s = wpool.tile([P, HPC, D], F32R)
        nc.gpsimd.dma_start(wo_s, t["woT"].rearrange("(h p) d -> p h d", p=P))
        cos_s = tabs.tile([P, NMT, HD], F32)
        nc.sync.dma_start(cos_s, t["cos"].rearrange("(m p) e -> p m e", p=P))
        sin_s = tabs.tile([P, NMT, HD], F32)
        nc.sync.dma_start(sin_s, t["sins"].rearrange("(m p) e -> p m e", p=P))

        for b in range(B):
            QT = qtkv.tile([P, HPC, S], F32R, tag="QT")
            KT = qtkv.tile([P, HPC, S], F32R, tag="KT")
            V = qtkv.tile([P, NMT, E2], F32R, tag="V")
            outT = outp.tile([P, HPC, S], F32R)

            # ---- phase A: projections + rope + transpose ----
            for mt in range(NMT):
                gm = b * S + mt * P
                xt = xtp.tile([P, ND, P], F32R)
                nc.gpsimd.dma_start(
                    xt, t["xT"][:, gm:gm + P].rearrange("(k p) m -> p k m", p=P)
                )
                qk_ps = psA.tile([P, E4], F32, tag="ps")
                nc.tensor.matmul(qk_ps, ones_r, bqk_s, start=True, stop=False)
                for k in range(ND):
                    nc.tensor.matmul(
                        qk_ps, xt[:, k, :], wqk_s[:, k, :],
                        start=False, stop=(k == ND - 1),
                    )
                v_ps = psA.tile([P, E2], F32, tag="ps")
                nc.tensor.matmul(v_ps, ones_r, bv_s, start=True, stop=False)
                for k in range(ND):
                    nc.tensor.matmul(
                        v_ps, xt[:, k, :], wv_s[:, k, :],
                        start=False, stop=(k == ND - 1),
                    )
                nc.vector.tensor_copy(V[:, mt, :], v_ps)

                qk = qkp.tile([P, E4], F32)
                u = up.tile([P, E4], F32)
                qk4 = qk.rearrange("p (g e) -> p g e", g=4)
                ps4 = qk_ps.rearrange("p (g e) -> p g e", g=4)
                u4 = u.rearrange("p (g e) -> p g e", g=4)
                cosm = cos_s[:, mt, :]
                sinm = sin_s[:, mt, :]
                H = HD // 2
                for g in range(4):
                    nc.vector.tensor_mul(qk4[:, g, :], ps4[:, g, :], cosm)
                for g in range(4):
                    nc.vector.tensor_mul(u4[:, g, 0:H], ps4[:, g, H:HD], sinm[:, 0:H])
                    nc.vector.tensor_mul(u4[:, g, H:HD], ps4[:, g, 0:H], sinm[:, H:HD])
                nc.vector.tensor_add(qk, qk, u)

                tp_ps = psA.tile([P, E4], F32, tag="ps")
                for g in range(4):
                    nc.tensor.transpose(
                        tp_ps[:, g * P:(g + 1) * P], qk4[:, g, :], ident
                    )
                msl = slice(mt * P, (mt + 1) * P)
                nc.vector.tensor_copy(
                    QT[:, :, msl], tp_ps[:, 0:E2].rearrange("p (h e) -> p h e", h=HPC)
                )
                nc.vector.tensor_copy(
                    KT[:, :, msl], tp_ps[:, E2:E4].rearrange("p (h e) -> p h e", h=HPC)
                )

            # ---- phase B: attention ----
            for h in range(HPC if "B" in phases else 0):
                for qc in range(NQC):
                    qsl = slice(qc * 512, (qc + 1) * 512)
                    av_ps = psAcc.tile([P, 512], F32)
                    l_ps = psL.tile([1, 512], F32)
                    for kt in range(NKT):
                        lg_ps = psA.tile([P, 512], F32, tag="ps")
                        nc.tensor.matmul(
                            lg_ps, KT[:, h, kt * P:(kt + 1) * P], QT[:, h, qsl],
                            start=True, stop=True,
                        )
                        at = attnp.tile([P, 512], F32R)
                        nc.scalar.activation(
                            at, lg_ps, mybir.ActivationFunctionType.Exp, scale=SCALE
                        )
                        nc.tensor.matmul(
                            av_ps, V[:, kt, h * HD:(h + 1) * HD], at,
                            start=(kt == 0), stop=(kt == NKT - 1),
                        )
                        nc.tensor.matmul(
                            l_ps, ones_c, at,
                            start=(kt == 0), stop=(kt == NKT - 1),
                        )
                    rs = rsp.tile([1, 512], F32R)
                    with nc.allow_low_precision(reason="f32r rounding of 1/l"):
                        nc.vector.reciprocal(rs, l_ps)
                    rb_ps = psA.tile([P, 512], F32, tag="ps")
                    nc.tensor.matmul(rb_ps, ones_r, rs, start=True, stop=True)
                    rb = rbp.tile([P, 512], F32)
                    nc.scalar.copy(rb, rb_ps)
                    nc.vector.tensor_mul(outT[:, h, qsl], av_ps, rb)

            # ---- phase C: output projection ----
            for mt in range(NMT if "C" in phases else 0):
                msl = slice(mt * P, (mt + 1) * P)
                for oc in range(4):
                    osl = slice(oc * 512, (oc + 1) * 512)
                    y_ps = psA.tile([P, 512], F32, tag="ps")
                    nc.tensor.matmul(
                        y_ps, outT[:, 0, msl], wo_s[:, 0, osl],
                        start=True, stop=False,
                    )
                    nc.tensor.matmul(
                        y_ps, outT[:, 1, msl], wo_s[:, 1, osl],
                        start=False, stop=True,
                    )
                    yt = yp.tile([P, 512], F32)
                    nc.scalar.copy(yt, y_ps)
                    nc.sync.dma_start(t["y"][b * S + mt * P: b * S + (mt + 1) * P, osl], yt)


def build_program():
    nc = bacc.Bacc(
        "TRN2",
        target_bir_lowering=False,
        debug=False,
        enable_asserts=False,
        num_devices=N_CORES,
    )
    t = {
        "xT": nc.dram_tensor("xT", [D, M], F32, kind="ExternalInput").ap(),
        "wqkT": nc.dram_tensor("wqkT", [D, E4], F32, kind="ExternalInput").ap(),
        "wvT": nc.dram_tensor("wvT", [D, E2], F32, kind="ExternalInput").ap(),
        "woT": nc.dram_tensor("woT", [E2, D], F32, kind="ExternalInput").ap(),
        "bqk": nc.dram_tensor("bqk", [1, E4], F32, kind="ExternalInput").ap(),
        "bv": nc.dram_tensor("bv", [1, E2], F32, kind="ExternalInput").ap(),
        "cos": nc.dram_tensor("cos", [S, HD], F32, kind="ExternalInput").ap(),
        "sins": nc.dram_tensor("sins", [S, HD], F32, kind="ExternalInput").ap(),
        "y": nc.dram_tensor("y", [M, D], F32, kind="ExternalOutput").ap(),
    }
    with tile.TileContext(nc) as tc:
        _emit(nc, tc, t)
    nc.compile()
    return nc


def rope_tables():
    inv_freq = 1.0 / (ROPE_THETA ** (np.arange(0, HD, 2, dtype=np.float32) / HD))
    angles = np.outer(np.arange(S, dtype=np.float32), inv_freq)
    ang = np.concatenate([angles, angles], axis=-1)
    cos = np.cos(ang).astype(np.float32)
    sin = np.sin(ang).astype(np.float32)
    sins = np.concatenate([-sin[:, :64], sin[:, 64:]], axis=-1)
    return cos, sins


def make_in_maps(x, wq, bq, wk, bk, wv, bv, wo, bo):
    xT = np.ascontiguousarray(x.reshape(M, D).T)
    cos, sins = rope_tables()
    maps = []
    for c in range(N_CORES):
        sl = slice(c * E2, (c + 1) * E2)
        maps.append({
            "xT": xT,
            "wqkT": np.ascontiguousarray(np.concatenate([wq[sl], wk[sl]], axis=0).T),
            "wvT": np.ascontiguousarray(wv[sl].T),
            "woT": np.ascontiguousarray(wo[:, sl].T),
            "bqk": np.concatenate([bq[sl], bk[sl]])[None, :].astype(np.float32),
            "bv": bv[sl][None, :].astype(np.float32),
            "cos": cos,
            "sins": sins,
        })
    return maps


_NC = None


def kernel(**inputs) -> np.ndarray:
    global _NC
    inputs = {k: np.ascontiguousarray(np.asarray(v, dtype=np.float32))
              for k, v in inputs.items()}
    if _NC is None:
        _NC = build_program()
    from concourse.bass_utils import run_bass_kernel_spmd

    maps = make_in_maps(**inputs)
    res = run_bass_kernel_spmd(_NC, maps, list(range(N_CORES)))
    y = np.zeros((M, D), np.float64)
    for c in range(N_CORES):
        y += res.results[c]["y"]
    y += inputs["bo"][None, :]
    return y.astype(np.float32).reshape(B, S, D)
